# revision 5
# baseline (speedup 1.0000x reference)
"""CRF loss (forward-algorithm denominator + gold-path numerator) on 8 Trainium2 cores.

v3 strategy (data-parallel over batch, 8 batch elements per core):
  The forward recursion runs in LINEAR space as a product of 512 per-step
  transition matrices E_t = exp(scores[t] - KAPPA), KAPPA = log(T)+0.5:
      v_512 = (E_1 E_2 ... E_512)^T e_START,  denominator = ln(v[END]) + S*KAPPA.

  Instead of a 512-step serial matrix-VECTOR scan (latency-bound, ~4.3us/step),
  the product is folded as chunked matrix-MATRIX prefix products:
    - 64 chunks x 8 steps: each (batch, chunk) chain folds its 8 fp8 matrices
      with 7 matmuls (round-0 rhs is itself a packed E slice).  All 512 chains
      per core are independent -> PE pipelines across 32-chain waves.
    - A 5-level pairing tree combines chunk products.  Stored orientation
      alternates by node parity (even nodes hold P^T, odd hold P), which makes
      every tree matmul consume stored tiles directly with zero transposes:
        natural  P   = mm(lhsT=X[2m]   (transposed-stored), rhs=X[2m+1])
        transposed P^T = mm(lhsT=X[2m+1] (natural-stored),  rhs=X[2m])
    - Final: denominator_b = ln( sum_j Xa[j,START] * Xb[j,END] ) + S*KAPPA
      where Xa = transposed-stored product of chunks 0..31, Xb = natural
      product of chunks 32..63.  Assembled via 4 strided [64,4] copies, one
      elementwise mult and a ones-vector matmul -> [8,1].

  dtypes: E stream + chunk chains fp8e4m3 (16.7 MB/core DMA), tree levels >=1
  stored bf16 (level-1 matmuls are fp8xfp8, upper levels bf16xbf16 -- no mixed
  dtype anywhere).  PSUM evacuation copies are split DVE/ACT per wave.

  numerator: host gathers the gold rows s[t,b,ti,:] into a [128,2048] bf16
  tile (pure index prep); device multiplies by the (j==tj)*mask one-hot and
  reduces, with a final selector matmul for the cross-partition per-batch sums.
"""
import math
import os
import numpy as np

S = 512
B = 64
T = 64
BQ = 8          # batch per core
N_CORES = 8
START_TAG = 62
END_TAG = 63
L = 8           # time steps per chunk
C = S // L      # 64 chunks
CPG = 4         # chunks per group (wave)
NG = C // CPG   # 16 groups
KAPPA = float(np.float32(math.log(T) + 0.5))

# copy split point (columns of the [128,1024] PSUM wave handled by DVE;
# remainder goes to the scalar/ACT engine)
CSPLIT = int(os.environ.get("KCSPLIT", "448"))

_COMPILED = None


def _build(hw_repeat=1):
    import concourse.bass as bass
    import concourse.bacc as bacc
    import concourse.mybir as mybir
    import concourse.tile as tile
    from concourse._compat import axon_active

    dt = mybir.dt
    AF = mybir.ActivationFunctionType
    ALU = mybir.AluOpType

    nc = bacc.Bacc(
        "TRN2", target_bir_lowering=False, debug=not axon_active(), num_devices=N_CORES
    )

    epack_d = nc.declare_dram_parameter(
        "epack", [NG, 128, 8 * 1024], dt.float8e4, isOutput=False
    )
    sel8_d = nc.declare_dram_parameter("sel8", [128, 8], dt.float32, isOutput=False)
    cbias_d = nc.declare_dram_parameter("cbias", [8, 1], dt.float32, isOutput=False)
    sgath_d = nc.declare_dram_parameter("sgath", [128, 32 * 64], dt.bfloat16, isOutput=False)
    eqm_d = nc.declare_dram_parameter("eqmask", [128, 32 * 64], dt.bfloat16, isOutput=False)
    loss_d = nc.declare_dram_parameter("loss", [BQ, 1], dt.float32, isOutput=True)

    def split_copy(dst, src):
        """Evacuate one PSUM wave to SBUF, split across DVE and ACT."""
        w = dst.shape[-1]
        cs = min(CSPLIT, w)
        nc.vector.tensor_copy(out=dst[:, 0:cs], in_=src[:, 0:cs])
        if cs < w:
            nc.scalar.copy(out=dst[:, cs:w], in_=src[:, cs:w])

    with tile.TileContext(nc) as tc:
        with (
            tc.tile_pool(name="static", bufs=1) as static_pool,
            tc.tile_pool(name="ering", bufs=3) as ering,
            tc.tile_pool(name="pout", bufs=3, space="PSUM") as pout_pool,
            tc.tile_pool(name="fin", bufs=1, space="PSUM") as fin_psum,
            tc.tile_pool(name="fins", bufs=1) as fin_sbuf,
        ):
            # ---- static tiles ----
            sel8 = static_pool.tile([128, 8], dt.float32)
            cbias = static_pool.tile([8, 1], dt.float32)
            ones128 = static_pool.tile([128, 1], dt.float32)
            zbias = static_pool.tile([128, 1], dt.float32)
            nc.vector.memset(ones128[:], 1.0)
            nc.vector.memset(zbias[:], 0.0)
            nc.sync.dma_start(out=sel8[:], in_=sel8_d[:])
            nc.sync.dma_start(out=cbias[:], in_=cbias_d[:])
            sgath = static_pool.tile([128, 32 * 64], dt.bfloat16)
            eqm = static_pool.tile([128, 32 * 64], dt.bfloat16)
            nprod = static_pool.tile([128, 32 * 64], dt.float32)
            npart = static_pool.tile([128, 1], dt.float32)
            nc.sync.dma_start(out=sgath[:], in_=sgath_d[:])
            nc.sync.dma_start(out=eqm[:], in_=eqm_d[:])

            # chunk-product tiles (one per group) and tree-level tiles
            qtiles = [
                static_pool.tile(
                    [128, CPG * 4 * 64], dt.float8e4, tag=f"q{g}", name=f"q{g}"
                )
                for g in range(NG)
            ]
            # level ell has NG >> (ell-1) tiles of 4 nodes (16 slots) each;
            # level 5 has one tile of 2 nodes (8 slots)
            lvtiles = {
                ell: [
                    static_pool.tile(
                        [128, (8 if ell == 5 else 16) * 64],
                        dt.bfloat16,
                        tag=f"lv{ell}_{tau}",
                        name=f"lv{ell}_{tau}",
                    )
                    for tau in range(max(1, NG >> ell))
                ]
                for ell in range(1, 6)
            }

            # ---- numerator (once; read-only inside the loop) ----
            numer = fin_psum.tile([8, 1], dt.float32, space="PSUM")
            nc.vector.tensor_tensor(out=nprod[:], in0=sgath[:], in1=eqm[:], op=ALU.mult)
            nc.vector.tensor_reduce(
                out=npart[:], in_=nprod[:], axis=mybir.AxisListType.X, op=ALU.add
            )
            nc.tensor.matmul(out=numer[:], lhsT=sel8[:], rhs=npart[:], start=True, stop=True)

            # ---- helpers ----
            def emit_group(g):
                """Phase 1: fold the 8 fp8 matrices of 32 chains (4 chunks x
                8 batches).  Round-0 rhs comes straight from the DMA tile."""
                et = ering.tile([128, 8 * 1024], dt.float8e4, tag="et")
                nc.sync.dma_start(out=et[:], in_=epack_d[g])
                qt = qtiles[g]
                for r in range(L - 1):
                    po = pout_pool.tile([128, 1024], dt.float32, tag="pout", space="PSUM")
                    for h in range(2):
                        for j in range(16):
                            lhsT = et[64 * h : 64 * h + 64,
                                      (r + 1) * 1024 + 64 * j : (r + 1) * 1024 + 64 * j + 64]
                            if r == 0:
                                rhs = et[64 * h : 64 * h + 64, 64 * j : 64 * j + 64]
                            else:
                                rhs = qt[64 * h : 64 * h + 64, 64 * j : 64 * j + 64]
                            nc.tensor.matmul(
                                out=po[64 * h : 64 * h + 64, 64 * j : 64 * j + 64],
                                lhsT=lhsT,
                                rhs=rhs,
                                start=True,
                                stop=True,
                            )
                    split_copy(qt[:], po[:])

            def child_ap(ell, n, h, bm):
                """AP of tree child node n (level ell-1) for batch (h, bm)."""
                if ell == 1:
                    t = qtiles[n // 4]
                else:
                    t = lvtiles[ell - 1][n // 4]
                col = 64 * (4 * (n % 4) + bm)
                return t[64 * h : 64 * h + 64, col : col + 64]

            def emit_tree_wave(ell, tau):
                """One tree wave: nodes m in [4tau, 4tau+4) (level 5: m in 0..1)
                for all 8 batches -> one PSUM tile, then evacuate."""
                ms = range(2) if ell == 5 else range(4 * tau, 4 * tau + 4)
                width = 8 * 64 if ell == 5 else 1024
                po = pout_pool.tile([128, width], dt.float32, tag="pout", space="PSUM")
                for m in ms:
                    for b in range(BQ):
                        h, bm = b // 4, b % 4
                        c0, c1 = 2 * m, 2 * m + 1
                        lt, rt = (c1, c0) if m % 2 == 0 else (c0, c1)
                        slot = 64 * (4 * (m % 4) + bm)
                        nc.tensor.matmul(
                            out=po[64 * h : 64 * h + 64, slot : slot + 64],
                            lhsT=child_ap(ell, lt, h, bm),
                            rhs=child_ap(ell, rt, h, bm),
                            start=True,
                            stop=True,
                        )
                split_copy(lvtiles[ell][tau][:], po[:])

            def emit_scan():
                for g in range(NG):
                    emit_group(g)
                    if g % 2 == 1:
                        emit_tree_wave(1, g // 2)
                    if g % 4 == 3:
                        emit_tree_wave(2, g // 4)
                    if g % 8 == 7:
                        emit_tree_wave(3, g // 8)
                emit_tree_wave(4, 0)
                emit_tree_wave(5, 0)

            if hw_repeat > 1:
                with tc.For_i(0, hw_repeat) as _i:
                    emit_scan()
            else:
                emit_scan()

            # ---- final assembly (outside the repeat loop, like the numerator) ----
            lv5 = lvtiles[5][0]
            colA = fin_sbuf.tile([128, 8], dt.float32)
            colB = fin_sbuf.tile([128, 8], dt.float32)
            nc.vector.memset(colA[:], 0.0)
            nc.vector.memset(colB[:], 0.0)
            # Xa at slots j=bm (m=0), Xb at slots j=4+bm (m=1); batch b=4h+bm
            for h in range(2):
                p0, p1 = 64 * h, 64 * h + 64
                cb0 = 4 * h
                sa, sb = START_TAG, 4 * 64 + END_TAG
                nc.vector.tensor_copy(
                    out=colA[p0:p1, cb0 : cb0 + 4],
                    in_=lv5[p0:p1, sa : sa + 3 * 64 + 1 : 64],
                )
                nc.vector.tensor_copy(
                    out=colB[p0:p1, cb0 : cb0 + 4],
                    in_=lv5[p0:p1, sb : sb + 3 * 64 + 1 : 64],
                )
            prod8 = fin_sbuf.tile([128, 8], dt.float32)
            nc.vector.tensor_tensor(out=prod8[:], in0=colA[:], in1=colB[:], op=ALU.mult)
            dps = fin_psum.tile([8, 1], dt.float32, space="PSUM")
            nc.tensor.matmul(out=dps[:], lhsT=prod8[:], rhs=ones128[:], start=True, stop=True)
            dlog = fin_sbuf.tile([8, 1], dt.float32)
            nc.scalar.activation(out=dlog[:], in_=dps[:], func=AF.Ln, bias=zbias[0:8])
            dmn = fin_sbuf.tile([8, 1], dt.float32)
            nc.vector.tensor_tensor(out=dmn[:], in0=dlog[:], in1=numer[:], op=ALU.subtract)
            dmc = fin_sbuf.tile([8, 1], dt.float32)
            nc.vector.tensor_tensor(out=dmc[:], in0=dmn[:], in1=cbias[:], op=ALU.add)
            lossv = fin_sbuf.tile([8, 1], dt.float32)
            nc.vector.tensor_scalar_mul(out=lossv[:], in0=dmc[:], scalar1=1.0 / B)
            nc.sync.dma_start(out=loss_d[:], in_=lossv[:])

    nc.compile()
    return nc


def _host_inputs(scores, target, mask):
    """Build per-core input maps. Batch q on core n = original batch 8n+q."""
    import ml_dtypes

    f8 = ml_dtypes.float8_e4m3
    scores = np.ascontiguousarray(scores, dtype=np.float32)
    target = np.asarray(target, dtype=np.int32)
    mask = np.asarray(mask, dtype=np.int32)

    E8 = np.exp(scores - KAPPA).astype(f8)  # (S, B, T, T)

    # block kblk of chunk c: time = c*L + (kblk if c even else L-1-kblk),
    # transposed iff (kblk==0) == (c even)
    cc_ = np.arange(C)[:, None]
    kb_ = np.arange(L)[None, :]
    tidx = cc_ * L + np.where(cc_ % 2 == 0, kb_, L - 1 - kb_)  # (C, L)
    trans = np.where(cc_ % 2 == 0, kb_ == 0, kb_ != 0)  # (C, L)

    blocks = E8[tidx]  # (C, L, B, T, T)
    blocks[trans] = blocks[trans].swapaxes(-1, -2)

    # -> epack[n, g, (h,p), (kblk, cc, bm, q)]
    bl = blocks.reshape(NG, CPG, L, N_CORES, 2, 4, T, T)
    epack = np.ascontiguousarray(
        bl.transpose(3, 0, 4, 6, 2, 1, 5, 7)
    ).reshape(N_CORES, NG, 128, L * 1024)

    sel8 = np.zeros((128, 8), dtype=np.float32)
    for q in range(BQ):
        sel8[q * 16 : q * 16 + 16, q] = 1.0
    cbias = np.full((8, 1), S * KAPPA, dtype=np.float32)

    ti = (target // T).astype(np.int64)  # (S, B)
    tj = (target % T).astype(np.int64)
    jr = np.arange(64)
    t_all = np.arange(S)

    in_maps = []
    for n in range(N_CORES):
        sgath = np.zeros((128, 32, 64), dtype=ml_dtypes.bfloat16)
        eqmask = np.zeros((128, 32, 64), dtype=ml_dtypes.bfloat16)
        for q in range(BQ):
            b = n * BQ + q
            p = q * 16 + (t_all % 16)
            nn = t_all // 16
            sgath[p, nn] = scores[t_all, b, ti[:, b]].astype(ml_dtypes.bfloat16)
            eqmask[p, nn] = (
                (jr[None, :] == tj[:, b][:, None]) * mask[:, b][:, None]
            ).astype(ml_dtypes.bfloat16)
        in_maps.append(
            {
                "epack": epack[n],
                "sel8": sel8,
                "cbias": cbias,
                "sgath": sgath.reshape(128, 32 * 64),
                "eqmask": eqmask.reshape(128, 32 * 64),
            }
        )
    return in_maps


def kernel(scores, target, mask):
    global _COMPILED
    from concourse.bass_utils import run_bass_kernel_spmd

    if _COMPILED is None:
        _COMPILED = _build()
    nc = _COMPILED
    in_maps = _host_inputs(scores, target, mask)
    res = run_bass_kernel_spmd(nc, in_maps, list(range(N_CORES)))

    loss = np.zeros(B, dtype=np.float32)
    for n in range(N_CORES):
        loss[n * BQ : (n + 1) * BQ] = res.results[n]["loss"].reshape(BQ)
    return loss


# revision 8
# speedup vs baseline: 1.1298x; 1.1298x over previous
"""CRF loss (forward-algorithm denominator + gold-path numerator) on 8 Trainium2 cores.

v3 strategy (data-parallel over batch, 8 batch elements per core):
  The forward recursion runs in LINEAR space as a product of 512 per-step
  transition matrices E_t = exp(scores[t] - KAPPA), KAPPA = log(T)+0.5:
      v_512 = (E_1 E_2 ... E_512)^T e_START,  denominator = ln(v[END]) + S*KAPPA.

  Instead of a 512-step serial matrix-VECTOR scan (latency-bound, ~4.3us/step),
  the product is folded as chunked matrix-MATRIX prefix products:
    - 64 chunks x 8 steps: each (batch, chunk) chain folds its 8 fp8 matrices
      with 7 matmuls (round-0 rhs is itself a packed E slice).  All 512 chains
      per core are independent -> PE pipelines across 32-chain waves.
    - A 5-level pairing tree combines chunk products.  Stored orientation
      alternates by node parity (even nodes hold P^T, odd hold P), which makes
      every tree matmul consume stored tiles directly with zero transposes:
        natural  P   = mm(lhsT=X[2m]   (transposed-stored), rhs=X[2m+1])
        transposed P^T = mm(lhsT=X[2m+1] (natural-stored),  rhs=X[2m])
    - Final: denominator_b = ln( sum_j Xa[j,START] * Xb[j,END] ) + S*KAPPA
      where Xa = transposed-stored product of chunks 0..31, Xb = natural
      product of chunks 32..63.  Assembled via 4 strided [64,4] copies, one
      elementwise mult and a ones-vector matmul -> [8,1].

  dtypes: E stream + chunk chains fp8e4m3 (16.7 MB/core DMA), tree levels >=1
  stored bf16 (level-1 matmuls are fp8xfp8, upper levels bf16xbf16 -- no mixed
  dtype anywhere).  PSUM evacuation copies are split DVE/ACT per wave.

  numerator: host gathers the gold rows s[t,b,ti,:] into a [128,2048] bf16
  tile (pure index prep); device multiplies by the (j==tj)*mask one-hot and
  reduces, with a final selector matmul for the cross-partition per-batch sums.
"""
import math
import os
import numpy as np

S = 512
B = 64
T = 64
BQ = 8          # batch per core
N_CORES = 8
START_TAG = 62
END_TAG = 63
L = 8           # time steps per chunk
C = S // L      # 64 chunks
CPG = 4         # chunks per group (wave)
NG = C // CPG   # 16 groups
KAPPA = float(np.float32(math.log(T) + 0.5))

# copy split point (columns of the [128,1024] PSUM wave handled by DVE;
# remainder goes to the scalar/ACT engine)
CSPLIT = int(os.environ.get("KCSPLIT", "448"))

_COMPILED = None


def _build(hw_repeat=1):
    import concourse.bass as bass
    import concourse.bacc as bacc
    import concourse.mybir as mybir
    import concourse.tile as tile
    from concourse._compat import axon_active

    dt = mybir.dt
    AF = mybir.ActivationFunctionType
    ALU = mybir.AluOpType

    nc = bacc.Bacc(
        "TRN2", target_bir_lowering=False, debug=not axon_active(), num_devices=N_CORES
    )

    epack_d = nc.declare_dram_parameter(
        "epack", [NG, 128, 8 * 1024], dt.float8e4, isOutput=False
    )
    sel8_d = nc.declare_dram_parameter("sel8", [128, 8], dt.float32, isOutput=False)
    cbias_d = nc.declare_dram_parameter("cbias", [8, 1], dt.float32, isOutput=False)
    sgath_d = nc.declare_dram_parameter("sgath", [128, 32 * 64], dt.bfloat16, isOutput=False)
    eqm_d = nc.declare_dram_parameter("eqmask", [128, 32 * 64], dt.bfloat16, isOutput=False)
    loss_d = nc.declare_dram_parameter("loss", [BQ, 1], dt.float32, isOutput=True)

    def split_copy(dst, src):
        """Evacuate one PSUM wave to SBUF, split across DVE and ACT."""
        w = dst.shape[-1]
        cs = min(CSPLIT, w)
        nc.vector.tensor_copy(out=dst[:, 0:cs], in_=src[:, 0:cs])
        if cs < w:
            nc.scalar.copy(out=dst[:, cs:w], in_=src[:, cs:w])

    with tile.TileContext(nc) as tc:
        with (
            tc.tile_pool(name="static", bufs=1) as static_pool,
            tc.tile_pool(name="ering", bufs=4) as ering,
            tc.tile_pool(name="pout", bufs=3, space="PSUM") as pout_pool,
            tc.tile_pool(name="fin", bufs=1, space="PSUM") as fin_psum,
            tc.tile_pool(name="fins", bufs=1) as fin_sbuf,
        ):
            # ---- static tiles ----
            sel8 = static_pool.tile([128, 8], dt.float32)
            cbias = static_pool.tile([8, 1], dt.float32)
            ones128 = static_pool.tile([128, 1], dt.float32)
            zbias = static_pool.tile([128, 1], dt.float32)
            nc.vector.memset(ones128[:], 1.0)
            nc.vector.memset(zbias[:], 0.0)
            nc.sync.dma_start(out=sel8[:], in_=sel8_d[:])
            nc.sync.dma_start(out=cbias[:], in_=cbias_d[:])
            sgath = static_pool.tile([128, 32 * 64], dt.bfloat16)
            eqm = static_pool.tile([128, 32 * 64], dt.bfloat16)
            nprod = static_pool.tile([128, 32 * 64], dt.float32)
            npart = static_pool.tile([128, 1], dt.float32)
            nc.sync.dma_start(out=sgath[:], in_=sgath_d[:])
            nc.sync.dma_start(out=eqm[:], in_=eqm_d[:])

            # chunk-product tiles (one per group) and tree-level tiles
            qtiles = [
                static_pool.tile(
                    [128, CPG * 4 * 64], dt.float8e4, tag=f"q{g}", name=f"q{g}"
                )
                for g in range(NG)
            ]
            # level ell has NG >> (ell-1) tiles of 4 nodes (16 slots) each;
            # level 5 has one tile of 2 nodes (8 slots)
            lvtiles = {
                ell: [
                    static_pool.tile(
                        [128, (8 if ell == 5 else 16) * 64],
                        dt.bfloat16,
                        tag=f"lv{ell}_{tau}",
                        name=f"lv{ell}_{tau}",
                    )
                    for tau in range(max(1, NG >> ell))
                ]
                for ell in range(1, 6)
            }

            # ---- numerator (once; read-only inside the loop) ----
            numer = fin_psum.tile([8, 1], dt.float32, space="PSUM")
            nc.vector.tensor_tensor(out=nprod[:], in0=sgath[:], in1=eqm[:], op=ALU.mult)
            nc.vector.tensor_reduce(
                out=npart[:], in_=nprod[:], axis=mybir.AxisListType.X, op=ALU.add
            )
            nc.tensor.matmul(out=numer[:], lhsT=sel8[:], rhs=npart[:], start=True, stop=True)

            # ---- helpers ----
            def emit_round(g, r, et):
                """One phase-1 wave: fold matrix r+1 into all 32 chains of
                group g.  Round-0 rhs comes straight from the DMA tile."""
                qt = qtiles[g]
                po = pout_pool.tile([128, 1024], dt.float32, tag="pout", space="PSUM")
                for h in range(2):
                    for j in range(16):
                        lhsT = et[64 * h : 64 * h + 64,
                                  (r + 1) * 1024 + 64 * j : (r + 1) * 1024 + 64 * j + 64]
                        if r == 0:
                            rhs = et[64 * h : 64 * h + 64, 64 * j : 64 * j + 64]
                        else:
                            rhs = qt[64 * h : 64 * h + 64, 64 * j : 64 * j + 64]
                        nc.tensor.matmul(
                            out=po[64 * h : 64 * h + 64, 64 * j : 64 * j + 64],
                            lhsT=lhsT,
                            rhs=rhs,
                            start=True,
                            stop=True,
                        )
                split_copy(qt[:], po[:])

            def child_ap(ell, n, h, bm):
                """AP of tree child node n (level ell-1) for batch (h, bm)."""
                if ell == 1:
                    t = qtiles[n // 4]
                else:
                    t = lvtiles[ell - 1][n // 4]
                col = 64 * (4 * (n % 4) + bm)
                return t[64 * h : 64 * h + 64, col : col + 64]

            def emit_tree_wave(ell, tau):
                """One tree wave: nodes m in [4tau, 4tau+4) (level 5: m in 0..1)
                for all 8 batches -> one PSUM tile, then evacuate."""
                ms = range(2) if ell == 5 else range(4 * tau, 4 * tau + 4)
                width = 8 * 64 if ell == 5 else 1024
                po = pout_pool.tile([128, width], dt.float32, tag="pout", space="PSUM")
                for m in ms:
                    for b in range(BQ):
                        h, bm = b // 4, b % 4
                        c0, c1 = 2 * m, 2 * m + 1
                        lt, rt = (c1, c0) if m % 2 == 0 else (c0, c1)
                        slot = 64 * (4 * (m % 4) + bm)
                        nc.tensor.matmul(
                            out=po[64 * h : 64 * h + 64, slot : slot + 64],
                            lhsT=child_ap(ell, lt, h, bm),
                            rhs=child_ap(ell, rt, h, bm),
                            start=True,
                            stop=True,
                        )
                split_copy(lvtiles[ell][tau][:], po[:])

            def emit_scan():
                """Interleave phase-1 waves across a window of IW groups so the
                in-order PE queue always holds work that is independent of the
                copy in flight; ready tree waves are slotted in as filler."""
                IW = 2
                ets = {}

                def ensure_dma(g):
                    if g < NG and g not in ets:
                        et = ering.tile([128, 8 * 1024], dt.float8e4, tag="et")
                        nc.sync.dma_start(out=et[:], in_=epack_d[g])
                        ets[g] = et

                pending = []
                for g in range(2 * IW):
                    ensure_dma(g)
                for w in range(NG // IW):
                    gs = [w * IW + k for k in range(IW)]
                    for g in gs:
                        ensure_dma(g + 2 * IW)
                    for r in range(L - 1):
                        for g in gs:
                            emit_round(g, r, ets[g])
                        if pending:
                            emit_tree_wave(*pending.pop(0))
                    for g in gs:
                        del ets[g]
                    # tree waves that became ready once group gs[-1] completed
                    gl = gs[-1]
                    if gl % 2 == 1:
                        pending.append((1, gl // 2))
                    if gl % 4 == 3:
                        pending.append((2, gl // 4))
                    if gl % 8 == 7:
                        pending.append((3, gl // 8))
                for ell, tau in pending:
                    emit_tree_wave(ell, tau)
                emit_tree_wave(4, 0)
                emit_tree_wave(5, 0)

            if hw_repeat > 1:
                with tc.For_i(0, hw_repeat) as _i:
                    emit_scan()
            else:
                emit_scan()

            # ---- final assembly (outside the repeat loop, like the numerator) ----
            lv5 = lvtiles[5][0]
            colA = fin_sbuf.tile([128, 8], dt.float32)
            colB = fin_sbuf.tile([128, 8], dt.float32)
            nc.vector.memset(colA[:], 0.0)
            nc.vector.memset(colB[:], 0.0)
            # Xa at slots j=bm (m=0), Xb at slots j=4+bm (m=1); batch b=4h+bm
            for h in range(2):
                p0, p1 = 64 * h, 64 * h + 64
                cb0 = 4 * h
                sa, sb = START_TAG, 4 * 64 + END_TAG
                nc.vector.tensor_copy(
                    out=colA[p0:p1, cb0 : cb0 + 4],
                    in_=lv5[p0:p1, sa : sa + 3 * 64 + 1 : 64],
                )
                nc.vector.tensor_copy(
                    out=colB[p0:p1, cb0 : cb0 + 4],
                    in_=lv5[p0:p1, sb : sb + 3 * 64 + 1 : 64],
                )
            prod8 = fin_sbuf.tile([128, 8], dt.float32)
            nc.vector.tensor_tensor(out=prod8[:], in0=colA[:], in1=colB[:], op=ALU.mult)
            dps = fin_psum.tile([8, 1], dt.float32, space="PSUM")
            nc.tensor.matmul(out=dps[:], lhsT=prod8[:], rhs=ones128[:], start=True, stop=True)
            dlog = fin_sbuf.tile([8, 1], dt.float32)
            nc.scalar.activation(out=dlog[:], in_=dps[:], func=AF.Ln, bias=zbias[0:8])
            dmn = fin_sbuf.tile([8, 1], dt.float32)
            nc.vector.tensor_tensor(out=dmn[:], in0=dlog[:], in1=numer[:], op=ALU.subtract)
            dmc = fin_sbuf.tile([8, 1], dt.float32)
            nc.vector.tensor_tensor(out=dmc[:], in0=dmn[:], in1=cbias[:], op=ALU.add)
            lossv = fin_sbuf.tile([8, 1], dt.float32)
            nc.vector.tensor_scalar_mul(out=lossv[:], in0=dmc[:], scalar1=1.0 / B)
            nc.sync.dma_start(out=loss_d[:], in_=lossv[:])

    nc.compile()
    return nc


def _host_inputs(scores, target, mask):
    """Build per-core input maps. Batch q on core n = original batch 8n+q."""
    import ml_dtypes

    f8 = ml_dtypes.float8_e4m3
    scores = np.ascontiguousarray(scores, dtype=np.float32)
    target = np.asarray(target, dtype=np.int32)
    mask = np.asarray(mask, dtype=np.int32)

    E8 = np.exp(scores - KAPPA).astype(f8)  # (S, B, T, T)

    # block kblk of chunk c: time = c*L + (kblk if c even else L-1-kblk),
    # transposed iff (kblk==0) == (c even)
    cc_ = np.arange(C)[:, None]
    kb_ = np.arange(L)[None, :]
    tidx = cc_ * L + np.where(cc_ % 2 == 0, kb_, L - 1 - kb_)  # (C, L)
    trans = np.where(cc_ % 2 == 0, kb_ == 0, kb_ != 0)  # (C, L)

    blocks = E8[tidx]  # (C, L, B, T, T)
    blocks[trans] = blocks[trans].swapaxes(-1, -2)

    # -> epack[n, g, (h,p), (kblk, cc, bm, q)]
    bl = blocks.reshape(NG, CPG, L, N_CORES, 2, 4, T, T)
    epack = np.ascontiguousarray(
        bl.transpose(3, 0, 4, 6, 2, 1, 5, 7)
    ).reshape(N_CORES, NG, 128, L * 1024)

    sel8 = np.zeros((128, 8), dtype=np.float32)
    for q in range(BQ):
        sel8[q * 16 : q * 16 + 16, q] = 1.0
    cbias = np.full((8, 1), S * KAPPA, dtype=np.float32)

    ti = (target // T).astype(np.int64)  # (S, B)
    tj = (target % T).astype(np.int64)
    jr = np.arange(64)
    t_all = np.arange(S)

    in_maps = []
    for n in range(N_CORES):
        sgath = np.zeros((128, 32, 64), dtype=ml_dtypes.bfloat16)
        eqmask = np.zeros((128, 32, 64), dtype=ml_dtypes.bfloat16)
        for q in range(BQ):
            b = n * BQ + q
            p = q * 16 + (t_all % 16)
            nn = t_all // 16
            sgath[p, nn] = scores[t_all, b, ti[:, b]].astype(ml_dtypes.bfloat16)
            eqmask[p, nn] = (
                (jr[None, :] == tj[:, b][:, None]) * mask[:, b][:, None]
            ).astype(ml_dtypes.bfloat16)
        in_maps.append(
            {
                "epack": epack[n],
                "sel8": sel8,
                "cbias": cbias,
                "sgath": sgath.reshape(128, 32 * 64),
                "eqmask": eqmask.reshape(128, 32 * 64),
            }
        )
    return in_maps


def kernel(scores, target, mask):
    global _COMPILED
    from concourse.bass_utils import run_bass_kernel_spmd

    if _COMPILED is None:
        _COMPILED = _build()
    nc = _COMPILED
    in_maps = _host_inputs(scores, target, mask)
    res = run_bass_kernel_spmd(nc, in_maps, list(range(N_CORES)))

    loss = np.zeros(B, dtype=np.float32)
    for n in range(N_CORES):
        loss[n * BQ : (n + 1) * BQ] = res.results[n]["loss"].reshape(BQ)
    return loss


# revision 27
# speedup vs baseline: 3.5133x; 3.1096x over previous
"""CRF loss (forward-algorithm denominator + gold-path numerator) on 8 Trainium2 cores.

v3 strategy (data-parallel over batch, 8 batch elements per core):
  The forward recursion runs in LINEAR space as a product of 512 per-step
  transition matrices E_t = exp(scores[t] - KAPPA), KAPPA = log(T)+0.5:
      v_512 = (E_1 E_2 ... E_512)^T e_START,  denominator = ln(v[END]) + S*KAPPA.

  Instead of a 512-step serial matrix-VECTOR scan (latency-bound, ~4.3us/step),
  the product is folded as chunked matrix-MATRIX prefix products:
    - 64 chunks x 8 steps: each (batch, chunk) chain folds its 8 fp8 matrices
      with 7 matmuls (round-0 rhs is itself a packed E slice).  All 512 chains
      per core are independent -> PE pipelines across 32-chain waves.
    - A 5-level pairing tree combines chunk products.  Stored orientation
      alternates by node parity (even nodes hold P^T, odd hold P), which makes
      every tree matmul consume stored tiles directly with zero transposes:
        natural  P   = mm(lhsT=X[2m]   (transposed-stored), rhs=X[2m+1])
        transposed P^T = mm(lhsT=X[2m+1] (natural-stored),  rhs=X[2m])
    - Final: denominator_b = ln( sum_j Xa[j,START] * Xb[j,END] ) + S*KAPPA
      where Xa = transposed-stored product of chunks 0..31, Xb = natural
      product of chunks 32..63.  Assembled via 4 strided [64,4] copies, one
      elementwise mult and a ones-vector matmul -> [8,1].

  dtypes: E stream + chunk chains fp8e4m3 (16.7 MB/core DMA), tree levels >=1
  stored bf16 (level-1 matmuls are fp8xfp8, upper levels bf16xbf16 -- no mixed
  dtype anywhere).  PSUM evacuation copies are split DVE/ACT per wave.

  numerator: host gathers the gold rows s[t,b,ti,:] into a [128,2048] bf16
  tile (pure index prep); device multiplies by the (j==tj)*mask one-hot and
  reduces, with a final selector matmul for the cross-partition per-batch sums.
"""
import math
import os
import numpy as np

S = 512
B = 64
T = 64
BQ = 8          # batch per core
N_CORES = 8
START_TAG = 62
END_TAG = 63
L = 8           # time steps per chunk
C = S // L      # 64 chunks
CPG = 4         # chunks per group (wave)
NG = C // CPG   # 16 groups
KAPPA = float(np.float32(math.log(T) + 0.5))

# copy split point (columns of the [128,1024] PSUM wave handled by DVE;
# remainder goes to the scalar/ACT engine)
CSPLIT = int(os.environ.get("KCSPLIT", "448"))
# ablation mode for perf localization: "", minicopy, minimm, nodma, empty
ABLATE = os.environ.get("KABLATE", "")
# 4-quadrant PE tiling for phase-1 waves
QUAD4 = os.environ.get("KQUAD4", "1") == "1"

_COMPILED = None


def _build(hw_repeat=1):
    import concourse.bass as bass
    import concourse.bacc as bacc
    import concourse.mybir as mybir
    import concourse.tile as tile
    from concourse._compat import axon_active

    dt = mybir.dt
    AF = mybir.ActivationFunctionType
    ALU = mybir.AluOpType

    nc = bacc.Bacc(
        "TRN2", target_bir_lowering=False, debug=not axon_active(), num_devices=N_CORES
    )

    epack_d = nc.declare_dram_parameter(
        "epack", [NG, 128, 8 * 1024], dt.float8e4, isOutput=False
    )
    sel8_d = nc.declare_dram_parameter("sel8", [128, 8], dt.float32, isOutput=False)
    cbias_d = nc.declare_dram_parameter("cbias", [8, 1], dt.float32, isOutput=False)
    sgath_d = nc.declare_dram_parameter("sgath", [128, 32 * 64], dt.bfloat16, isOutput=False)
    eqm_d = nc.declare_dram_parameter("eqmask", [128, 32 * 64], dt.bfloat16, isOutput=False)
    loss_d = nc.declare_dram_parameter("loss", [BQ, 1], dt.float32, isOutput=True)

    def split_copy(dst, src):
        """Evacuate one PSUM wave to SBUF, split across DVE and ACT."""
        w = dst.shape[-1]
        if ABLATE == "minicopy":
            nc.vector.tensor_copy(out=dst[:, 0:64], in_=src[:, 0:64])
            return
        cs = min(CSPLIT, w)
        nc.vector.tensor_copy(out=dst[:, 0:cs], in_=src[:, 0:cs])
        if cs < w:
            nc.scalar.copy(out=dst[:, cs:w], in_=src[:, cs:w])

    with tile.TileContext(nc) as tc:
        with (
            tc.tile_pool(name="static", bufs=1) as static_pool,
            tc.tile_pool(name="ering", bufs=4) as ering,
            tc.tile_pool(name="pout", bufs=3, space="PSUM") as pout_pool,
            tc.tile_pool(name="fin", bufs=1, space="PSUM") as fin_psum,
            tc.tile_pool(name="fins", bufs=1) as fin_sbuf,
        ):
            # ---- static tiles ----
            sel8 = static_pool.tile([128, 8], dt.float32)
            cbias = static_pool.tile([8, 1], dt.float32)
            ones128 = static_pool.tile([128, 1], dt.float32)
            zbias = static_pool.tile([128, 1], dt.float32)
            nc.vector.memset(ones128[:], 1.0)
            nc.vector.memset(zbias[:], 0.0)
            nc.sync.dma_start(out=sel8[:], in_=sel8_d[:])
            nc.sync.dma_start(out=cbias[:], in_=cbias_d[:])
            sgath = static_pool.tile([128, 32 * 64], dt.bfloat16)
            eqm = static_pool.tile([128, 32 * 64], dt.bfloat16)
            nprod = static_pool.tile([128, 32 * 64], dt.float32)
            npart = static_pool.tile([128, 1], dt.float32)
            nc.sync.dma_start(out=sgath[:], in_=sgath_d[:])
            nc.sync.dma_start(out=eqm[:], in_=eqm_d[:])

            # chunk-product tiles (one per group) and tree-level tiles
            qtiles = [
                static_pool.tile(
                    [128, CPG * 4 * 64], dt.float8e4, tag=f"q{g}", name=f"q{g}"
                )
                for g in range(NG)
            ]
            # level ell has NG >> (ell-1) tiles of 4 nodes (16 slots) each;
            # level 5 has one tile of 2 nodes (8 slots)
            lvtiles = {
                ell: [
                    static_pool.tile(
                        [128, (8 if ell == 5 else 16) * 64],
                        dt.bfloat16,
                        tag=f"lv{ell}_{tau}",
                        name=f"lv{ell}_{tau}",
                    )
                    for tau in range(max(1, NG >> ell))
                ]
                for ell in range(1, 6)
            }

            coljunk = static_pool.tile([128, 8], dt.float32)

            # ---- numerator (once; read-only inside the loop) ----
            numer = fin_psum.tile([8, 1], dt.float32, space="PSUM")
            nc.vector.tensor_tensor(out=nprod[:], in0=sgath[:], in1=eqm[:], op=ALU.mult)
            nc.vector.tensor_reduce(
                out=npart[:], in_=nprod[:], axis=mybir.AxisListType.X, op=ALU.add
            )
            nc.tensor.matmul(out=numer[:], lhsT=sel8[:], rhs=npart[:], start=True, stop=True)

            # ---- helpers ----
            def emit_round(g, r, et):
                """One phase-1 wave: fold matrix r+1 into all 32 chains of
                group g.  Round-0 rhs comes straight from the DMA tile."""
                qt = qtiles[g]
                mw = 8 if ABLATE == "minimm" else 64
                po = pout_pool.tile([128, 1024], dt.float32, tag="pout", space="PSUM")
                for h in range(2):
                    for j in range(16):
                        # 4-quad: odd slots read the opposite half (hi); PSUM
                        # banks are segregated by row tile (bank hi) so no two
                        # row tiles touch the same bank+partition group, and
                        # qtiles permanently store this segregated layout
                        hi = (h ^ (j & 1)) if QUAD4 else h
                        oc = (512 * hi + 64 * (j // 2)) if QUAD4 else 64 * j
                        rc = (512 * h + 64 * (j // 2)) if QUAD4 else 64 * j
                        lhsT = et[64 * hi : 64 * hi + 64,
                                  (r + 1) * 1024 + 64 * j : (r + 1) * 1024 + 64 * j + mw]
                        if r == 0:
                            rhs = et[64 * hi : 64 * hi + 64, 64 * j : 64 * j + mw]
                        else:
                            rhs = qt[64 * hi : 64 * hi + 64, rc : rc + mw]
                        nc.tensor.matmul(
                            out=po[64 * h : 64 * h + mw, oc : oc + mw],
                            lhsT=lhsT,
                            rhs=rhs,
                            start=True,
                            stop=True,
                        )
                split_copy(qt[:], po[:])

            def child_ap(ell, n, h, bm):
                """AP of tree child node n (level ell-1) for batch (h, bm)."""
                if ell == 1:
                    t = qtiles[n // 4]
                    if QUAD4:
                        col = 512 * (h ^ (bm & 1)) + 64 * (2 * (n % 4) + bm // 2)
                    else:
                        col = 64 * (4 * (n % 4) + bm)
                else:
                    t = lvtiles[ell - 1][n // 4]
                    col = 64 * (4 * (n % 4) + bm)
                return t[64 * h : 64 * h + 64, col : col + 64]

            def emit_tree_wave(ell, tau):
                """One tree wave: nodes m in [4tau, 4tau+4) (level 5: m in 0..1)
                for all 8 batches -> one PSUM tile, then evacuate."""
                ms = range(2) if ell == 5 else range(4 * tau, 4 * tau + 4)
                width = 8 * 64 if ell == 5 else 1024
                po = pout_pool.tile([128, width], dt.float32, tag="pout", space="PSUM")
                for m in ms:
                    for b in range(BQ):
                        h, bm = b // 4, b % 4
                        c0, c1 = 2 * m, 2 * m + 1
                        lt, rt = (c1, c0) if m % 2 == 0 else (c0, c1)
                        slot = 64 * (4 * (m % 4) + bm)
                        nc.tensor.matmul(
                            out=po[64 * h : 64 * h + 64, slot : slot + 64],
                            lhsT=child_ap(ell, lt, h, bm),
                            rhs=child_ap(ell, rt, h, bm),
                            start=True,
                            stop=True,
                        )
                split_copy(lvtiles[ell][tau][:], po[:])

            def emit_scan():
                """Interleave phase-1 waves across a window of IW groups so the
                in-order PE queue always holds work that is independent of the
                copy in flight; ready tree waves are slotted in as filler."""
                IW = 2
                ets = {}
                if ABLATE == "empty":
                    nc.vector.memset(coljunk[:], 0.0)
                    return

                def ensure_dma(g):
                    if g < NG and g not in ets:
                        et = ering.tile([128, 8 * 1024], dt.float8e4, tag="et")
                        if ABLATE == "minidma":
                            nc.sync.dma_start(out=et[:, 0:128], in_=epack_d[g][:, 0:128])
                        elif ABLATE != "nodma":
                            nc.sync.dma_start(out=et[:], in_=epack_d[g])
                        ets[g] = et

                pending = []
                for g in range(2 * IW):
                    ensure_dma(g)
                for w in range(NG // IW):
                    gs = [w * IW + k for k in range(IW)]
                    for g in gs:
                        ensure_dma(g + 2 * IW)
                    for r in range(L - 1):
                        for g in gs:
                            emit_round(g, r, ets[g])
                        if pending:
                            emit_tree_wave(*pending.pop(0))
                    for g in gs:
                        del ets[g]
                    # tree waves that became ready once group gs[-1] completed
                    gl = gs[-1]
                    if gl % 2 == 1:
                        pending.append((1, gl // 2))
                    if gl % 4 == 3:
                        pending.append((2, gl // 4))
                    if gl % 8 == 7:
                        pending.append((3, gl // 8))
                for ell, tau in pending:
                    emit_tree_wave(ell, tau)
                emit_tree_wave(4, 0)
                emit_tree_wave(5, 0)

            if hw_repeat > 1:
                hints = (
                    mybir.EngineType.PE,
                    mybir.EngineType.DVE,
                    mybir.EngineType.Activation,
                    mybir.EngineType.SP,
                )
                with tc.For_i(0, hw_repeat, hint_engines=hints) as _i:
                    emit_scan()
            else:
                emit_scan()

            # ---- final assembly (outside the repeat loop, like the numerator) ----
            lv5 = lvtiles[5][0]
            colA = fin_sbuf.tile([128, 8], dt.float32)
            colB = fin_sbuf.tile([128, 8], dt.float32)
            nc.vector.memset(colA[:], 0.0)
            nc.vector.memset(colB[:], 0.0)
            # Xa at slots j=bm (m=0), Xb at slots j=4+bm (m=1); batch b=4h+bm
            for h in range(2):
                p0, p1 = 64 * h, 64 * h + 64
                cb0 = 4 * h
                sa, sb = START_TAG, 4 * 64 + END_TAG
                nc.vector.tensor_copy(
                    out=colA[p0:p1, cb0 : cb0 + 4],
                    in_=lv5[p0:p1, sa : sa + 3 * 64 + 1 : 64],
                )
                nc.vector.tensor_copy(
                    out=colB[p0:p1, cb0 : cb0 + 4],
                    in_=lv5[p0:p1, sb : sb + 3 * 64 + 1 : 64],
                )
            prod8 = fin_sbuf.tile([128, 8], dt.float32)
            nc.vector.tensor_tensor(out=prod8[:], in0=colA[:], in1=colB[:], op=ALU.mult)
            dps = fin_psum.tile([8, 1], dt.float32, space="PSUM")
            nc.tensor.matmul(out=dps[:], lhsT=prod8[:], rhs=ones128[:], start=True, stop=True)
            dlog = fin_sbuf.tile([8, 1], dt.float32)
            nc.scalar.activation(out=dlog[:], in_=dps[:], func=AF.Ln, bias=zbias[0:8])
            dmn = fin_sbuf.tile([8, 1], dt.float32)
            nc.vector.tensor_tensor(out=dmn[:], in0=dlog[:], in1=numer[:], op=ALU.subtract)
            dmc = fin_sbuf.tile([8, 1], dt.float32)
            nc.vector.tensor_tensor(out=dmc[:], in0=dmn[:], in1=cbias[:], op=ALU.add)
            lossv = fin_sbuf.tile([8, 1], dt.float32)
            nc.vector.tensor_scalar_mul(out=lossv[:], in0=dmc[:], scalar1=1.0 / B)
            nc.sync.dma_start(out=loss_d[:], in_=lossv[:])

    nc.compile()
    return nc


def _host_inputs(scores, target, mask):
    """Build per-core input maps. Batch q on core n = original batch 8n+q."""
    import ml_dtypes

    f8 = ml_dtypes.float8_e4m3
    scores = np.ascontiguousarray(scores, dtype=np.float32)
    target = np.asarray(target, dtype=np.int32)
    mask = np.asarray(mask, dtype=np.int32)

    E8 = np.exp(scores - KAPPA).astype(f8)  # (S, B, T, T)

    # block kblk of chunk c: time = c*L + (kblk if c even else L-1-kblk),
    # transposed iff (kblk==0) == (c even)
    cc_ = np.arange(C)[:, None]
    kb_ = np.arange(L)[None, :]
    tidx = cc_ * L + np.where(cc_ % 2 == 0, kb_, L - 1 - kb_)  # (C, L)
    trans = np.where(cc_ % 2 == 0, kb_ == 0, kb_ != 0)  # (C, L)

    blocks = E8[tidx]  # (C, L, B, T, T)
    blocks[trans] = blocks[trans].swapaxes(-1, -2)

    # -> epack[n, g, (h,p), (kblk, cc, bm, q)]
    bl = blocks.reshape(NG, CPG, L, N_CORES, 2, 4, T, T)
    epack = np.ascontiguousarray(
        bl.transpose(3, 0, 4, 6, 2, 1, 5, 7)
    ).reshape(N_CORES, NG, 128, L * 1024)
    if QUAD4:
        # odd-slot chains alternate partition halves each round: init (kblk 0)
        # and even-round weights (odd kblk) are packed on the opposite half
        epv = epack.reshape(N_CORES, NG, 2, 64, L, CPG, 4, 64)
        for kb in range(L):
            if kb == 0 or kb % 2 == 1:
                epv[:, :, :, :, kb, :, 1::2, :] = (
                    epv[:, :, ::-1, :, kb, :, 1::2, :].copy()
                )

    sel8 = np.zeros((128, 8), dtype=np.float32)
    for q in range(BQ):
        sel8[q * 16 : q * 16 + 16, q] = 1.0
    cbias = np.full((8, 1), S * KAPPA, dtype=np.float32)

    ti = (target // T).astype(np.int64)  # (S, B)
    tj = (target % T).astype(np.int64)
    jr = np.arange(64)
    t_all = np.arange(S)

    in_maps = []
    for n in range(N_CORES):
        sgath = np.zeros((128, 32, 64), dtype=ml_dtypes.bfloat16)
        eqmask = np.zeros((128, 32, 64), dtype=ml_dtypes.bfloat16)
        for q in range(BQ):
            b = n * BQ + q
            p = q * 16 + (t_all % 16)
            nn = t_all // 16
            sgath[p, nn] = scores[t_all, b, ti[:, b]].astype(ml_dtypes.bfloat16)
            eqmask[p, nn] = (
                (jr[None, :] == tj[:, b][:, None]) * mask[:, b][:, None]
            ).astype(ml_dtypes.bfloat16)
        in_maps.append(
            {
                "epack": epack[n],
                "sel8": sel8,
                "cbias": cbias,
                "sgath": sgath.reshape(128, 32 * 64),
                "eqmask": eqmask.reshape(128, 32 * 64),
            }
        )
    return in_maps


def kernel(scores, target, mask):
    global _COMPILED
    from concourse.bass_utils import run_bass_kernel_spmd

    if _COMPILED is None:
        _COMPILED = _build()
    nc = _COMPILED
    in_maps = _host_inputs(scores, target, mask)
    res = run_bass_kernel_spmd(nc, in_maps, list(range(N_CORES)))

    loss = np.zeros(B, dtype=np.float32)
    for n in range(N_CORES):
        loss[n * BQ : (n + 1) * BQ] = res.results[n]["loss"].reshape(BQ)
    return loss


# revision 30
# speedup vs baseline: 3.9950x; 1.1371x over previous
"""CRF loss (forward-algorithm denominator + gold-path numerator) on 8 Trainium2 cores.

v3 strategy (data-parallel over batch, 8 batch elements per core):
  The forward recursion runs in LINEAR space as a product of 512 per-step
  transition matrices E_t = exp(scores[t] - KAPPA), KAPPA = log(T)+0.5:
      v_512 = (E_1 E_2 ... E_512)^T e_START,  denominator = ln(v[END]) + S*KAPPA.

  Instead of a 512-step serial matrix-VECTOR scan (latency-bound, ~4.3us/step),
  the product is folded as chunked matrix-MATRIX prefix products:
    - 64 chunks x 8 steps: each (batch, chunk) chain folds its 8 fp8 matrices
      with 7 matmuls (round-0 rhs is itself a packed E slice).  All 512 chains
      per core are independent -> PE pipelines across 32-chain waves.
    - A 5-level pairing tree combines chunk products.  Stored orientation
      alternates by node parity (even nodes hold P^T, odd hold P), which makes
      every tree matmul consume stored tiles directly with zero transposes:
        natural  P   = mm(lhsT=X[2m]   (transposed-stored), rhs=X[2m+1])
        transposed P^T = mm(lhsT=X[2m+1] (natural-stored),  rhs=X[2m])
    - Final: denominator_b = ln( sum_j Xa[j,START] * Xb[j,END] ) + S*KAPPA
      where Xa = transposed-stored product of chunks 0..31, Xb = natural
      product of chunks 32..63.  Assembled via 4 strided [64,4] copies, one
      elementwise mult and a ones-vector matmul -> [8,1].

  dtypes: E stream + chunk chains fp8e4m3 (16.7 MB/core DMA), tree levels >=1
  stored bf16 (level-1 matmuls are fp8xfp8, upper levels bf16xbf16 -- no mixed
  dtype anywhere).  PSUM evacuation copies are split DVE/ACT per wave.

  numerator: host gathers the gold rows s[t,b,ti,:] into a [128,2048] bf16
  tile (pure index prep); device multiplies by the (j==tj)*mask one-hot and
  reduces, with a final selector matmul for the cross-partition per-batch sums.
"""
import math
import os
import numpy as np

S = 512
B = 64
T = 64
BQ = 8          # batch per core
N_CORES = 8
START_TAG = 62
END_TAG = 63
L = 8           # time steps per chunk
C = S // L      # 64 chunks
CPG = int(os.environ.get("KCPG", "8"))  # chunks per group (wave)
NG = C // CPG   # 16 groups
KAPPA = float(np.float32(math.log(T) + 0.5))

# copy split point (columns of the [128,1024] PSUM wave handled by DVE;
# remainder goes to the scalar/ACT engine)
CSPLIT = int(os.environ.get("KCSPLIT", "448"))
# ablation mode for perf localization: "", minicopy, minimm, nodma, empty
ABLATE = os.environ.get("KABLATE", "")
# 4-quadrant PE tiling for phase-1 waves
QUAD4 = os.environ.get("KQUAD4", "1") == "1"

_COMPILED = None


def _build(hw_repeat=1):
    import concourse.bass as bass
    import concourse.bacc as bacc
    import concourse.mybir as mybir
    import concourse.tile as tile
    from concourse._compat import axon_active

    dt = mybir.dt
    AF = mybir.ActivationFunctionType
    ALU = mybir.AluOpType

    nc = bacc.Bacc(
        "TRN2", target_bir_lowering=False, debug=not axon_active(), num_devices=N_CORES
    )

    epack_d = nc.declare_dram_parameter(
        "epack", [NG, 128, L * CPG * 4 * 64], dt.float8e4, isOutput=False
    )
    sel8_d = nc.declare_dram_parameter("sel8", [128, 8], dt.float32, isOutput=False)
    cbias_d = nc.declare_dram_parameter("cbias", [8, 1], dt.float32, isOutput=False)
    sgath_d = nc.declare_dram_parameter("sgath", [128, 32 * 64], dt.bfloat16, isOutput=False)
    eqm_d = nc.declare_dram_parameter("eqmask", [128, 32 * 64], dt.bfloat16, isOutput=False)
    loss_d = nc.declare_dram_parameter("loss", [BQ, 1], dt.float32, isOutput=True)

    def split_copy(dst, src):
        """Evacuate one PSUM wave to SBUF, split across DVE and ACT."""
        w = dst.shape[-1]
        if ABLATE == "minicopy":
            nc.vector.tensor_copy(out=dst[:, 0:64], in_=src[:, 0:64])
            return
        cs = min(CSPLIT, w)
        nc.vector.tensor_copy(out=dst[:, 0:cs], in_=src[:, 0:cs])
        if cs < w:
            nc.scalar.copy(out=dst[:, cs:w], in_=src[:, cs:w])

    with tile.TileContext(nc) as tc:
        with (
            tc.tile_pool(name="static", bufs=1) as static_pool,
            tc.tile_pool(name="ering", bufs=4) as ering,
            tc.tile_pool(name="pout", bufs=2, space="PSUM") as pout_pool,
            tc.tile_pool(name="fins", bufs=1) as fin_sbuf,
        ):
            # ---- static tiles ----
            sel8 = static_pool.tile([128, 8], dt.float32)
            cbias = static_pool.tile([8, 1], dt.float32)
            ones128 = static_pool.tile([128, 1], dt.float32)
            zbias = static_pool.tile([128, 1], dt.float32)
            nc.vector.memset(ones128[:], 1.0)
            nc.vector.memset(zbias[:], 0.0)
            nc.sync.dma_start(out=sel8[:], in_=sel8_d[:])
            nc.sync.dma_start(out=cbias[:], in_=cbias_d[:])
            sgath = static_pool.tile([128, 32 * 64], dt.bfloat16)
            eqm = static_pool.tile([128, 32 * 64], dt.bfloat16)
            nprod = static_pool.tile([128, 32 * 64], dt.float32)
            npart = static_pool.tile([128, 1], dt.float32)
            nc.sync.dma_start(out=sgath[:], in_=sgath_d[:])
            nc.sync.dma_start(out=eqm[:], in_=eqm_d[:])

            # chunk-product tiles (one per group) and tree-level tiles
            qtiles = [
                static_pool.tile(
                    [128, CPG * 4 * 64], dt.float8e4, tag=f"q{g}", name=f"q{g}"
                )
                for g in range(NG)
            ]
            # level ell has NG >> (ell-1) tiles of 4 nodes (16 slots) each;
            # level 5 has one tile of 2 nodes (8 slots)
            lvtiles = {
                ell: [
                    static_pool.tile(
                        [128, (8 if ell == 5 else 16) * 64],
                        dt.bfloat16,
                        tag=f"lv{ell}_{tau}",
                        name=f"lv{ell}_{tau}",
                    )
                    for tau in range(max(1, C >> (ell + 2)))
                ]
                for ell in range(1, 6)
            }

            coljunk = static_pool.tile([128, 8], dt.float32)

            # ---- numerator (once; read-only inside the loop, held in SBUF
            # so PSUM banks stay free for the big waves) ----
            numer_ps = pout_pool.tile([8, 1], dt.float32, tag="pout", space="PSUM")
            numer = static_pool.tile([8, 1], dt.float32)
            nc.vector.tensor_tensor(out=nprod[:], in0=sgath[:], in1=eqm[:], op=ALU.mult)
            nc.vector.tensor_reduce(
                out=npart[:], in_=nprod[:], axis=mybir.AxisListType.X, op=ALU.add
            )
            nc.tensor.matmul(out=numer_ps[:], lhsT=sel8[:], rhs=npart[:], start=True, stop=True)
            nc.vector.tensor_copy(out=numer[:], in_=numer_ps[:])

            # ---- helpers ----
            def emit_round(g, r, et):
                """One phase-1 wave: fold matrix r+1 into all 32 chains of
                group g.  Round-0 rhs comes straight from the DMA tile."""
                qt = qtiles[g]
                mw = 8 if ABLATE == "minimm" else 64
                HW = CPG * 2 * 64  # half-width: columns per row-tile bank set
                po = pout_pool.tile([128, 2 * HW], dt.float32, tag="pout", space="PSUM")
                for h in range(2):
                    for j in range(4 * CPG):
                        # 4-quad: odd slots read the opposite half (hi); PSUM
                        # banks are segregated by row tile (bank hi) so no two
                        # row tiles touch the same bank+partition group, and
                        # qtiles permanently store this segregated layout
                        hi = (h ^ (j & 1)) if QUAD4 else h
                        oc = (HW * hi + 64 * (j // 2)) if QUAD4 else 64 * j
                        rc = (HW * h + 64 * (j // 2)) if QUAD4 else 64 * j
                        lhsT = et[64 * hi : 64 * hi + 64,
                                  (r + 1) * 4 * CPG * 64 + 64 * j : (r + 1) * 4 * CPG * 64 + 64 * j + mw]
                        if r == 0:
                            rhs = et[64 * hi : 64 * hi + 64, 64 * j : 64 * j + mw]
                        else:
                            rhs = qt[64 * hi : 64 * hi + 64, rc : rc + mw]
                        nc.tensor.matmul(
                            out=po[64 * h : 64 * h + mw, oc : oc + mw],
                            lhsT=lhsT,
                            rhs=rhs,
                            start=True,
                            stop=True,
                        )
                split_copy(qt[:], po[:])

            def child_ap(ell, n, h, bm):
                """AP of tree child node n (level ell-1) for batch (h, bm)."""
                if ell == 1:
                    t = qtiles[n // CPG]
                    if QUAD4:
                        col = CPG * 2 * 64 * (h ^ (bm & 1)) + 64 * (2 * (n % CPG) + bm // 2)
                    else:
                        col = 64 * (4 * (n % CPG) + bm)
                else:
                    t = lvtiles[ell - 1][n // 4]
                    col = 64 * (4 * (n % 4) + bm)
                return t[64 * h : 64 * h + 64, col : col + 64]

            def emit_tree_wave(ell, tau):
                """One tree wave: nodes m in [4tau, 4tau+4) (level 5: m in 0..1)
                for all 8 batches -> one PSUM tile, then evacuate."""
                ms = range(2) if ell == 5 else range(4 * tau, 4 * tau + 4)
                width = 8 * 64 if ell == 5 else 1024
                po = pout_pool.tile([128, width], dt.float32, tag="pout", space="PSUM")
                for m in ms:
                    for b in range(BQ):
                        h, bm = b // 4, b % 4
                        c0, c1 = 2 * m, 2 * m + 1
                        lt, rt = (c1, c0) if m % 2 == 0 else (c0, c1)
                        slot = 64 * (4 * (m % 4) + bm)
                        nc.tensor.matmul(
                            out=po[64 * h : 64 * h + 64, slot : slot + 64],
                            lhsT=child_ap(ell, lt, h, bm),
                            rhs=child_ap(ell, rt, h, bm),
                            start=True,
                            stop=True,
                        )
                split_copy(lvtiles[ell][tau][:], po[:])

            def emit_scan():
                """Interleave phase-1 waves across a window of IW groups so the
                in-order PE queue always holds work that is independent of the
                copy in flight; ready tree waves are slotted in as filler."""
                IW = 2
                ets = {}
                if ABLATE == "empty":
                    nc.vector.memset(coljunk[:], 0.0)
                    return

                def ensure_dma(g):
                    if g < NG and g not in ets:
                        et = ering.tile([128, L * CPG * 4 * 64], dt.float8e4, tag="et")
                        if ABLATE == "minidma":
                            nc.sync.dma_start(out=et[:, 0:128], in_=epack_d[g][:, 0:128])
                        elif ABLATE != "nodma":
                            nc.sync.dma_start(out=et[:], in_=epack_d[g])
                        ets[g] = et

                pending = []
                next_tau = {1: 0, 2: 0, 3: 0}
                for g in range(2 * IW):
                    ensure_dma(g)
                for w in range(NG // IW):
                    gs = [w * IW + k for k in range(IW)]
                    for g in gs:
                        ensure_dma(g + 2 * IW)
                    for r in range(L - 1):
                        for g in gs:
                            emit_round(g, r, ets[g])
                        if pending:
                            emit_tree_wave(*pending.pop(0))
                    for g in gs:
                        del ets[g]
                    # tree waves that became ready once group gs[-1] completed:
                    # level-ell tile tau needs the first (tau+1)*2^(ell+2) chunks
                    done = (gs[-1] + 1) * CPG
                    for ell in (1, 2, 3):
                        while next_tau[ell] * (1 << (ell + 2)) + (1 << (ell + 2)) <= done:
                            pending.append((ell, next_tau[ell]))
                            next_tau[ell] += 1
                for ell, tau in pending:
                    emit_tree_wave(ell, tau)
                emit_tree_wave(4, 0)
                emit_tree_wave(5, 0)

            if hw_repeat > 1:
                hints = (
                    mybir.EngineType.PE,
                    mybir.EngineType.DVE,
                    mybir.EngineType.Activation,
                    mybir.EngineType.SP,
                )
                with tc.For_i(0, hw_repeat, hint_engines=hints) as _i:
                    emit_scan()
            else:
                emit_scan()

            # ---- final assembly (outside the repeat loop, like the numerator) ----
            lv5 = lvtiles[5][0]
            colA = fin_sbuf.tile([128, 8], dt.float32)
            colB = fin_sbuf.tile([128, 8], dt.float32)
            nc.vector.memset(colA[:], 0.0)
            nc.vector.memset(colB[:], 0.0)
            # Xa at slots j=bm (m=0), Xb at slots j=4+bm (m=1); batch b=4h+bm
            for h in range(2):
                p0, p1 = 64 * h, 64 * h + 64
                cb0 = 4 * h
                sa, sb = START_TAG, 4 * 64 + END_TAG
                nc.vector.tensor_copy(
                    out=colA[p0:p1, cb0 : cb0 + 4],
                    in_=lv5[p0:p1, sa : sa + 3 * 64 + 1 : 64],
                )
                nc.vector.tensor_copy(
                    out=colB[p0:p1, cb0 : cb0 + 4],
                    in_=lv5[p0:p1, sb : sb + 3 * 64 + 1 : 64],
                )
            prod8 = fin_sbuf.tile([128, 8], dt.float32)
            nc.vector.tensor_tensor(out=prod8[:], in0=colA[:], in1=colB[:], op=ALU.mult)
            dps = pout_pool.tile([8, 1], dt.float32, tag="pout", space="PSUM")
            nc.tensor.matmul(out=dps[:], lhsT=prod8[:], rhs=ones128[:], start=True, stop=True)
            dlog = fin_sbuf.tile([8, 1], dt.float32)
            nc.scalar.activation(out=dlog[:], in_=dps[:], func=AF.Ln, bias=zbias[0:8])
            dmn = fin_sbuf.tile([8, 1], dt.float32)
            nc.vector.tensor_tensor(out=dmn[:], in0=dlog[:], in1=numer[:], op=ALU.subtract)
            dmc = fin_sbuf.tile([8, 1], dt.float32)
            nc.vector.tensor_tensor(out=dmc[:], in0=dmn[:], in1=cbias[:], op=ALU.add)
            lossv = fin_sbuf.tile([8, 1], dt.float32)
            nc.vector.tensor_scalar_mul(out=lossv[:], in0=dmc[:], scalar1=1.0 / B)
            nc.sync.dma_start(out=loss_d[:], in_=lossv[:])

    nc.compile()
    return nc


def _host_inputs(scores, target, mask):
    """Build per-core input maps. Batch q on core n = original batch 8n+q."""
    import ml_dtypes

    f8 = ml_dtypes.float8_e4m3
    scores = np.ascontiguousarray(scores, dtype=np.float32)
    target = np.asarray(target, dtype=np.int32)
    mask = np.asarray(mask, dtype=np.int32)

    E8 = np.exp(scores - KAPPA).astype(f8)  # (S, B, T, T)

    # block kblk of chunk c: time = c*L + (kblk if c even else L-1-kblk),
    # transposed iff (kblk==0) == (c even)
    cc_ = np.arange(C)[:, None]
    kb_ = np.arange(L)[None, :]
    tidx = cc_ * L + np.where(cc_ % 2 == 0, kb_, L - 1 - kb_)  # (C, L)
    trans = np.where(cc_ % 2 == 0, kb_ == 0, kb_ != 0)  # (C, L)

    blocks = E8[tidx]  # (C, L, B, T, T)
    blocks[trans] = blocks[trans].swapaxes(-1, -2)

    # -> epack[n, g, (h,p), (kblk, cc, bm, q)]
    bl = blocks.reshape(NG, CPG, L, N_CORES, 2, 4, T, T)
    epack = np.ascontiguousarray(
        bl.transpose(3, 0, 4, 6, 2, 1, 5, 7)
    ).reshape(N_CORES, NG, 128, L * CPG * 4 * 64)
    if QUAD4:
        # odd-slot chains alternate partition halves each round: init (kblk 0)
        # and even-round weights (odd kblk) are packed on the opposite half
        epv = epack.reshape(N_CORES, NG, 2, 64, L, CPG, 4, 64)
        for kb in range(L):
            if kb == 0 or kb % 2 == 1:
                epv[:, :, :, :, kb, :, 1::2, :] = (
                    epv[:, :, ::-1, :, kb, :, 1::2, :].copy()
                )

    sel8 = np.zeros((128, 8), dtype=np.float32)
    for q in range(BQ):
        sel8[q * 16 : q * 16 + 16, q] = 1.0
    cbias = np.full((8, 1), S * KAPPA, dtype=np.float32)

    ti = (target // T).astype(np.int64)  # (S, B)
    tj = (target % T).astype(np.int64)
    jr = np.arange(64)
    t_all = np.arange(S)

    in_maps = []
    for n in range(N_CORES):
        sgath = np.zeros((128, 32, 64), dtype=ml_dtypes.bfloat16)
        eqmask = np.zeros((128, 32, 64), dtype=ml_dtypes.bfloat16)
        for q in range(BQ):
            b = n * BQ + q
            p = q * 16 + (t_all % 16)
            nn = t_all // 16
            sgath[p, nn] = scores[t_all, b, ti[:, b]].astype(ml_dtypes.bfloat16)
            eqmask[p, nn] = (
                (jr[None, :] == tj[:, b][:, None]) * mask[:, b][:, None]
            ).astype(ml_dtypes.bfloat16)
        in_maps.append(
            {
                "epack": epack[n],
                "sel8": sel8,
                "cbias": cbias,
                "sgath": sgath.reshape(128, 32 * 64),
                "eqmask": eqmask.reshape(128, 32 * 64),
            }
        )
    return in_maps


def kernel(scores, target, mask):
    global _COMPILED
    from concourse.bass_utils import run_bass_kernel_spmd

    if _COMPILED is None:
        _COMPILED = _build()
    nc = _COMPILED
    in_maps = _host_inputs(scores, target, mask)
    res = run_bass_kernel_spmd(nc, in_maps, list(range(N_CORES)))

    loss = np.zeros(B, dtype=np.float32)
    for n in range(N_CORES):
        loss[n * BQ : (n + 1) * BQ] = res.results[n]["loss"].reshape(BQ)
    return loss
